# revision 35
# baseline (speedup 1.0000x reference)
"""Self-contained Trainium2 kernel for nn_BatchGraphNuc (radius-graph build).

Reference computation (per full batch B=4096, N_ELEC=64, N_NUC=16, F=128):
    diff[b,i,j,:] = coord_elec[b,j,:] - coord_nuc[i,:]        # [B,16,64,3]
    dist          = ||diff||_2                                 # [B,16,64]
    mask          = dist < 5.0                                 # bool
    s_flat        = s_nuc.reshape(B*16, 128)                   # pure reshape
    v_flat        = v_nuc.reshape(B*16, 3, 128)                # pure reshape
    edge_index    = [b*16+i, b*64+j] for all (b,i,j)           # input-independent
    returns (s_flat, v_flat, edge_index, diff.reshape(-1,3), mask.reshape(-1))

Strategy: data-parallel over the batch dim across 8 NeuronCores (512
samples/core).  The device computes the two value-dependent outputs
(edge_attr = diff, mask); s_flat/v_flat are zero-copy host reshapes and
edge_index is a host iota.  mask is computed as ss < T where
ss = (dx^2 + dy^2) + dz^2 in the reference's f32 summation order and
T = 0x41C7FFFF (largest f32 with sqrt_f32(T) >= 5.0), which makes the
compare bit-equivalent to (sqrt_f32(ss) < 5.0) under correctly rounded
f32 sqrt.

Per-core schedule (engine balance, from the instruction cost model):
  DVE:    interleaved subtracts (the edge_attr tiles), compares, and the
          summation adds for the last pieces
  ACT:    elementwise squares
  GpSimd: summation adds for the early pieces
  DMA:    ~7.2 MB/core -> ~20 us at 358 GB/s is the floor; the big
          edge_attr writes stream back-to-back behind the subtract chain.
Work is cut into pieces along the nuc axis (chunk 0 finer so the DMA-out
stream starts early), all emitted so no engine's in-order queue blocks.
"""

import sys

if "/opt/trn_rl_repo" not in sys.path:
    sys.path.insert(0, "/opt/trn_rl_repo")

import numpy as np

B, N_ELEC, N_NUC, F = 4096, 64, 16, 128
N_CORES = 8
B_LOC = B // N_CORES          # 512 samples per core
P = 128                        # SBUF partitions
N_CHUNKS = B_LOC // P          # 4 chunks of 128 samples
E = N_NUC * N_ELEC             # 1024 edges per sample

# Largest f32 t such that all f32 x < t have round(sqrt(x)) < 5.0.
SS_LT_THRESHOLD = float(np.uint32(0x41C7FFFF).view(np.float32))

TRACE = False                  # test.py sets True to capture a HW profile
LAST_RESULT = None             # BassKernelResults of the last device run

# Tuning knobs (exercised by the sweep in tune.py; defaults = best found).
# SPLITS[t] = number of equal nuc-axis pieces for chunk t.
CONFIG = {
    "splits": ((2, 2, 4, 4, 4), 4, 4, 2),
    "n_dve_add_pieces": 2,     # adds of the last K pieces run on DVE
    "mask_per_chunk": True,    # batch mask DMA per chunk (4 DMAs, not 8)
    "in_queues": "nuc_gpsimd", # nuc via SWDGE, elec0 on SP, rest on ACT
    "mask_queue": "sp",        # engine queue for mask DMA-outs
}

_CACHE = {}


def _build_nc():
    import concourse.mybir as mybir
    from concourse import bacc, tile
    from concourse.bass import ts

    nc = bacc.Bacc("TRN2", target_bir_lowering=False, debug=False)

    ce = nc.dram_tensor(
        "coord_elec", [B_LOC, N_ELEC * 3], mybir.dt.float32, kind="ExternalInput"
    )
    cn = nc.dram_tensor(
        "coord_nuc", [N_NUC * 3], mybir.dt.float32, kind="ExternalInput"
    )
    ea = nc.dram_tensor(
        "edge_attr", [B_LOC, E * 3], mybir.dt.float32, kind="ExternalOutput"
    )
    mk = nc.dram_tensor("mask", [B_LOC, E], mybir.dt.uint8, kind="ExternalOutput")

    splits = CONFIG["splits"]
    # pieces: (chunk, nuc_start, nuc_count).  splits[t] is either an int
    # (split chunk t into that many equal nuc-axis pieces) or a tuple of
    # nuc counts summing to N_NUC (uneven pieces).
    pieces = []
    for t in range(N_CHUNKS):
        sp = splits[t]
        counts = [N_NUC // sp] * sp if isinstance(sp, int) else list(sp)
        assert sum(counts) == N_NUC
        i0 = 0
        for c in counts:
            pieces.append((t, i0, c))
            i0 += c
    n_pieces = len(pieces)
    dve_add = set(range(n_pieces - CONFIG["n_dve_add_pieces"], n_pieces))

    in_engines = {
        "act": [nc.scalar] * 5,
        "gpsimd": [nc.gpsimd] * 5,
        "sp": [nc.sync] * 5,
        "split": [nc.scalar, nc.scalar, nc.sync, nc.sync, nc.sync],
        "split2": [nc.scalar, nc.sync, nc.scalar, nc.scalar, nc.scalar],
        "split3": [nc.sync, nc.sync, nc.scalar, nc.scalar, nc.scalar],
        "nuc_gpsimd": [nc.gpsimd, nc.sync, nc.scalar, nc.scalar, nc.scalar],
        "gp_sp": [nc.gpsimd, nc.gpsimd, nc.scalar, nc.scalar, nc.scalar],
        "gp1": [nc.gpsimd, nc.sync, nc.gpsimd, nc.gpsimd, nc.gpsimd],
    }[CONFIG["in_queues"]]
    eaq = CONFIG.get("ea_queues", "sp")
    if eaq.startswith("gp_first"):
        k = int(eaq.rsplit("_", 1)[1])
        ea_engine = lambda p: nc.gpsimd if p < k else nc.sync
    else:
        pat = {
            "sp": [nc.sync],
            "act_sp": [nc.scalar, nc.sync],
            "sp_act": [nc.sync, nc.scalar],
        }[eaq]
        ea_engine = lambda p: pat[p % len(pat)]
    mask_engine = {"act": nc.scalar, "sp": nc.sync, "gpsimd": nc.gpsimd}[
        CONFIG["mask_queue"]
    ]

    with tile.TileContext(nc) as tc:
        with (
            tc.tile_pool(name="const", bufs=1) as const,
            tc.tile_pool(name="elecp", bufs=N_CHUNKS) as elecp,
            tc.tile_pool(name="work", bufs=1) as work,
        ):
            # The first subtract waits on max(nuc, elec0) — emission order
            # of those two loads is a tuning knob.
            nuc_t = const.tile([P, N_NUC * 3], mybir.dt.float32)
            elecs = []
            if CONFIG.get("elec0_first"):
                elec_t = elecp.tile([P, N_ELEC * 3], mybir.dt.float32)
                in_engines[1].dma_start(elec_t[:], ce[ts(0, P)])
                elecs.append(elec_t)
                in_engines[0].dma_start(nuc_t[:], cn[:].partition_broadcast(P))
            else:
                in_engines[0].dma_start(nuc_t[:], cn[:].partition_broadcast(P))
            if CONFIG.get("merge_elec") and len(elecs) <= 1:
                if not elecs:
                    elec_t = elecp.tile([P, N_ELEC * 3], mybir.dt.float32)
                    in_engines[1].dma_start(elec_t[:], ce[ts(0, P)])
                    elecs.append(elec_t)
                # chunks 1-3 in one DMA: [p, (t, jc)] <- ce rows 128..511
                nrest = N_CHUNKS - 1
                elec_big = elecp.tile(
                    [P, nrest * N_ELEC * 3], mybir.dt.float32, name="elec_big"
                )
                src = ce[P:, :].rearrange("(t p) jc -> p t jc", p=P)
                in_engines[2].dma_start(
                    elec_big[:].rearrange("p (t jc) -> p t jc", t=nrest), src
                )
                for t in range(1, N_CHUNKS):
                    elecs.append(
                        elec_big[:, (t - 1) * N_ELEC * 3 : t * N_ELEC * 3]
                    )
            else:
                for t in range(len(elecs), N_CHUNKS):
                    elec_t = elecp.tile([P, N_ELEC * 3], mybir.dt.float32)
                    in_engines[t + 1].dma_start(elec_t[:], ce[ts(t, P)])
                    elecs.append(elec_t)

            # Pass 1: all DVE subtracts + edge_attr DMA-outs, in piece
            # order.  Each piece has its own tile slot, so DVE streams the
            # subtracts back-to-back and the big DMA-outs follow.
            diffs = []
            for (t, i0, ncnt) in pieces:
                diff_t = work.tile(
                    [P, ncnt * N_ELEC * 3], mybir.dt.float32,
                    name=f"diff{len(diffs)}",
                )
                e_view = (
                    elecs[t][:]
                    .rearrange("p (j c) -> p j c", c=3)
                    .unsqueeze(1)
                    .to_broadcast((P, ncnt, N_ELEC, 3))
                )
                n_view = (
                    nuc_t[:, i0 * 3 : (i0 + ncnt) * 3]
                    .rearrange("p (i c) -> p i c", c=3)
                    .unsqueeze(2)
                    .to_broadcast((P, ncnt, N_ELEC, 3))
                )
                d_view = diff_t[:].rearrange(
                    "p (i j c) -> p i j c", i=ncnt, j=N_ELEC
                )
                nc.vector.tensor_tensor(
                    out=d_view, in0=e_view, in1=n_view,
                    op=mybir.AluOpType.subtract,
                )
                ea_engine(len(diffs)).dma_start(
                    ea[ts(t, P), i0 * N_ELEC * 3 : (i0 + ncnt) * N_ELEC * 3],
                    diff_t[:],
                )
                diffs.append(diff_t)

            # Pass 2a: squares on ACT, in piece order.
            sqs = []
            for p, (t, i0, ncnt) in enumerate(pieces):
                sq_t = work.tile(
                    [P, ncnt * N_ELEC * 3], mybir.dt.float32, name=f"sq{p}"
                )
                nc.scalar.activation(
                    out=sq_t[:], in_=diffs[p][:],
                    func=mybir.ActivationFunctionType.Square,
                )
                sqs.append(sq_t)

            # Pass 2b: ss = (x^2 + y^2) + z^2 — two adds per piece, in the
            # reference's left-to-right f32 order.  Early pieces go to the
            # otherwise idle GpSimd engine; the last K pieces go to DVE
            # (idle once the subtracts finish), emitted before any compare
            # so they cannot head-of-line-block DVE's in-order queue.
            sss = [None] * n_pieces

            def emit_adds(p, eng):
                (t, i0, ncnt) = pieces[p]
                ne = ncnt * N_ELEC
                sq_v = sqs[p][:].rearrange("p (e c) -> p e c", c=3)
                t1_t = work.tile([P, ne], mybir.dt.float32, name=f"t1_{p}")
                eng.tensor_tensor(
                    out=t1_t[:], in0=sq_v[:, :, 0], in1=sq_v[:, :, 1],
                    op=mybir.AluOpType.add,
                )
                ss_t = work.tile([P, ne], mybir.dt.float32, name=f"ss{p}")
                eng.tensor_tensor(
                    out=ss_t[:], in0=t1_t[:], in1=sq_v[:, :, 2],
                    op=mybir.AluOpType.add,
                )
                sss[p] = ss_t

            for p in range(n_pieces):
                if p not in dve_add:
                    emit_adds(p, nc.gpsimd)
            for p in range(n_pieces):
                if p in dve_add:
                    emit_adds(p, nc.vector)

            # Pass 2c: compares on DVE + mask DMA-outs.  With
            # mask_per_chunk, pieces of one chunk write disjoint column
            # ranges of a shared tile and a single DMA ships the chunk.
            # mask_group>1 merges that many chunks into one DMA (3D DRAM
            # access pattern), trading later readiness for fewer dispatch
            # and completion-semaphore overheads.
            mgroups = CONFIG.get("mask_groups")
            mg = CONFIG.get("mask_group", 1)
            if CONFIG["mask_per_chunk"] and mgroups:
                # Arbitrary contiguous chunk groups, e.g. [[0,1,2],[3]]:
                # early-ready masks merge into one mid-stream DMA; fewer
                # trailing completion semaphores at the kernel tail.
                grp_of = {}
                for gi, grp in enumerate(mgroups):
                    for t in grp:
                        grp_of[t] = (gi, grp[0], len(grp))
                grp_tiles = {}
                done_in_grp = {gi: 0 for gi in range(len(mgroups))}
                for p, (t, i0, ncnt) in enumerate(pieces):
                    gi, t0, glen = grp_of[t]
                    if gi not in grp_tiles:
                        grp_tiles[gi] = work.tile(
                            [P, glen * E], mybir.dt.uint8, name=f"mask_g{gi}"
                        )
                    mk_t = grp_tiles[gi]
                    tg = t - t0
                    nc.vector.tensor_scalar(
                        out=mk_t[:, tg * E + i0 * N_ELEC
                                 : tg * E + (i0 + ncnt) * N_ELEC],
                        in0=sss[p][:],
                        scalar1=SS_LT_THRESHOLD,
                        scalar2=None,
                        op0=mybir.AluOpType.is_lt,
                    )
                    if i0 + ncnt == N_NUC:
                        done_in_grp[gi] += 1
                    if done_in_grp[gi] == glen and i0 + ncnt == N_NUC:
                        dst = mk[t0 * P : (t0 + glen) * P, :].rearrange(
                            "(t p) e -> p t e", p=P
                        )
                        mask_engine.dma_start(
                            dst,
                            mk_t[:].rearrange("p (t e) -> p t e", t=glen),
                        )
            elif CONFIG["mask_per_chunk"] and mg > 1:
                group_tiles = {}
                mk_v3 = mk[:].rearrange("(g t p) e -> g p t e", t=mg, p=P)
                for p, (t, i0, ncnt) in enumerate(pieces):
                    g, tg = divmod(t, mg)
                    if g not in group_tiles:
                        group_tiles[g] = work.tile(
                            [P, mg * E], mybir.dt.uint8, name=f"mask_grp{g}"
                        )
                    mk_t = group_tiles[g]
                    nc.vector.tensor_scalar(
                        out=mk_t[:, tg * E + i0 * N_ELEC
                                 : tg * E + (i0 + ncnt) * N_ELEC],
                        in0=sss[p][:],
                        scalar1=SS_LT_THRESHOLD,
                        scalar2=None,
                        op0=mybir.AluOpType.is_lt,
                    )
                    if tg == mg - 1 and i0 + ncnt == N_NUC:
                        mask_engine.dma_start(
                            mk_v3[g],
                            mk_t[:].rearrange("p (t e) -> p t e", t=mg),
                        )
            elif CONFIG["mask_per_chunk"]:
                chunk_tiles = {}
                for p, (t, i0, ncnt) in enumerate(pieces):
                    if t not in chunk_tiles:
                        chunk_tiles[t] = work.tile(
                            [P, E], mybir.dt.uint8, name=f"mask_chunk{t}"
                        )
                    mk_t = chunk_tiles[t]
                    nc.vector.tensor_scalar(
                        out=mk_t[:, i0 * N_ELEC : (i0 + ncnt) * N_ELEC],
                        in0=sss[p][:],
                        scalar1=SS_LT_THRESHOLD,
                        scalar2=None,
                        op0=mybir.AluOpType.is_lt,
                    )
                    if i0 + ncnt == N_NUC:
                        mask_engine.dma_start(mk[ts(t, P)], mk_t[:])
            else:
                for p, (t, i0, ncnt) in enumerate(pieces):
                    ne = ncnt * N_ELEC
                    mk_t = work.tile([P, ne], mybir.dt.uint8, name=f"mk{p}")
                    nc.vector.tensor_scalar(
                        out=mk_t[:],
                        in0=sss[p][:],
                        scalar1=SS_LT_THRESHOLD,
                        scalar2=None,
                        op0=mybir.AluOpType.is_lt,
                    )
                    mask_engine.dma_start(
                        mk[ts(t, P), i0 * N_ELEC : (i0 + ncnt) * N_ELEC],
                        mk_t[:],
                    )

    nc.compile()
    return nc


def _get_nc():
    if "nc" not in _CACHE:
        _CACHE["nc"] = _build_nc()
    return _CACHE["nc"]


def _edge_index():
    if "edge_index" not in _CACHE:
        b = np.arange(B, dtype=np.int32)
        i = np.arange(N_NUC, dtype=np.int32)
        j = np.arange(N_ELEC, dtype=np.int32)
        row = (b[:, None, None] * N_NUC + i[None, :, None]) + np.zeros(
            (1, 1, N_ELEC), np.int32
        )
        col = (b[:, None, None] * N_ELEC + j[None, None, :]) + np.zeros(
            (1, N_NUC, 1), np.int32
        )
        _CACHE["edge_index"] = np.stack([row.reshape(-1), col.reshape(-1)])
    return _CACHE["edge_index"]


def _build_fast_path():
    """Persistent jitted SPMD executable mirroring bass2jax.run_bass_via_pjrt
    (same HLO → warm XLA caches), with device-resident zero output buffers so
    repeat calls skip the host->device upload of the donated outputs."""
    import jax
    import jax.numpy as jnp
    from jax.sharding import Mesh, NamedSharding, PartitionSpec

    from jax.experimental.shard_map import shard_map
    from concourse import mybir
    from concourse.bass2jax import (
        _bass_exec_p,
        install_neuronx_cc_hook,
        partition_id_tensor,
    )

    nc = _get_nc()
    install_neuronx_cc_hook()

    partition_name = (
        nc.partition_id_tensor.name if nc.partition_id_tensor else None
    )
    in_names, out_names, out_avals, zero_outs = [], [], [], []
    for alloc in nc.m.functions[0].allocations:
        if not isinstance(alloc, mybir.MemoryLocationSet):
            continue
        name = alloc.memorylocations[0].name
        if alloc.kind == "ExternalInput":
            if name != partition_name:
                in_names.append(name)
        elif alloc.kind == "ExternalOutput":
            out_names.append(name)
            shape = tuple(alloc.tensor_shape)
            dtype = mybir.dt.np(alloc.dtype)
            out_avals.append(jax.core.ShapedArray(shape, dtype))
            zero_outs.append(np.zeros(shape, dtype))
    n_params, n_outs = len(in_names), len(out_avals)
    all_in = in_names + out_names + ([partition_name] if partition_name else [])

    def _body(*args):
        operands = list(args)
        if partition_name is not None:
            operands.append(partition_id_tensor())
        return tuple(
            _bass_exec_p.bind(
                *operands,
                out_avals=tuple(out_avals),
                in_names=tuple(all_in),
                out_names=tuple(out_names),
                lowering_input_output_aliases=(),
                sim_require_finite=True,
                sim_require_nnan=True,
                nc=nc,
            )
        )

    devices = jax.devices()[:N_CORES]
    mesh = Mesh(np.asarray(devices), ("core",))
    spec = PartitionSpec("core")
    fn = jax.jit(
        shard_map(
            _body,
            mesh=mesh,
            in_specs=(spec,) * (n_params + n_outs),
            out_specs=(spec,) * n_outs,
            check_rep=False,
        ),
        donate_argnums=tuple(range(n_params, n_params + n_outs)),
        keep_unused=True,
    )
    sharding = NamedSharding(mesh, spec)
    zeros_dev = [
        jax.device_put(np.zeros((N_CORES * z.shape[0], *z.shape[1:]), z.dtype),
                       sharding)
        for z in zero_outs
    ]

    def run(ce_global, cn_global):
        ins = {"coord_elec": ce_global, "coord_nuc": cn_global}
        in_arrs = [ins[n] for n in in_names]
        # Donation consumes the buffers; hand jit cheap on-device copies.
        donated = [jnp.copy(z) for z in zeros_dev]
        outs = fn(*in_arrs, *donated)
        return {n: np.asarray(outs[i]) for i, n in enumerate(out_names)}

    return run


def kernel(s_nuc, v_nuc, coord_elec, coord_nuc):
    global LAST_RESULT
    from concourse.bass_utils import run_bass_kernel_spmd

    s_nuc = np.asarray(s_nuc)
    v_nuc = np.asarray(v_nuc)
    coord_elec = np.asarray(coord_elec, dtype=np.float32)
    coord_nuc = np.asarray(coord_nuc, dtype=np.float32)

    nc = _get_nc()

    ce_flat = np.ascontiguousarray(coord_elec.reshape(B, N_ELEC * 3))
    cn_flat = np.ascontiguousarray(coord_nuc.reshape(N_NUC * 3))

    use_fast = _CACHE.get("ran_once") and not TRACE and not _CACHE.get("fast_broken")
    if use_fast:
        # Warm path: persistent executable, device-resident zero buffers.
        try:
            if "fast" not in _CACHE:
                _CACHE["fast"] = _build_fast_path()
            cn_global = np.ascontiguousarray(
                np.broadcast_to(cn_flat, (N_CORES, N_NUC * 3))
            ).reshape(-1)
            outs = _CACHE["fast"](ce_flat, cn_global)
            edge_attr = outs["edge_attr"].reshape(B * E, 3)
            mask = outs["mask"].reshape(B * E).view(np.bool_)
        except Exception:
            _CACHE["fast_broken"] = True
            use_fast = False
    if not use_fast:
        in_maps = [
            {
                "coord_elec": ce_flat[c * B_LOC : (c + 1) * B_LOC],
                "coord_nuc": cn_flat,
            }
            for c in range(N_CORES)
        ]
        # Retries with backoff: the axon-tunneled device occasionally
        # reports a transient NRT error; a fresh attempt recovers.
        import time as _time

        last_exc = None
        for attempt, delay in enumerate((0, 2, 8)):
            if delay:
                _time.sleep(delay)
            try:
                res = run_bass_kernel_spmd(
                    nc,
                    in_maps,
                    core_ids=list(range(N_CORES)),
                    trace=TRACE and attempt == 0,
                )
                break
            except Exception as e:
                last_exc = e
        else:
            raise last_exc
        LAST_RESULT = res
        _CACHE["ran_once"] = True
        edge_attr = np.concatenate(
            [res.results[c]["edge_attr"] for c in range(N_CORES)], axis=0
        ).reshape(B * E, 3)
        mask = (
            np.concatenate(
                [res.results[c]["mask"] for c in range(N_CORES)], axis=0
            )
            .reshape(B * E)
            .view(np.bool_)
        )

    s_flat = s_nuc.reshape(B * N_NUC, F)
    v_flat = v_nuc.reshape(B * N_NUC, 3, F)
    return (s_flat, v_flat, _edge_index(), edge_attr, mask)
